# revision 7
# baseline (speedup 1.0000x reference)
"""Multi-head attention (B=2, S=2048, D=1024, H=16) on 8 Trainium2 cores.

Sharding: core c handles batch b = c//4 and head group g = c%4 (4 heads,
256 of the 1024 QKV output columns).

v2 design (ACT-bound target):
  - QKV projections in bf16 (lhsT = W.T column slice, rhs = x.T windows of
    512). q/k psums are evicted on DVE as fp8(e4m3) with the bias folded in
    (tensor_scalar_add), PLUS an fp8 residual slab for q computed in one
    scalar_tensor_tensor: r8 = (psum + bias) - q8. k8 is written twice
    (two identical slabs).
  - QK logits use the fp8 DoubleRow perf mode: one matmul per head per
    (i, j) tile contracts both slabs at 0.5 cycles/row:
    logitsT = k8.T @ q8 + k8.T @ r8 = k8.T @ (q8 + r8) — q at ~fp16
    precision, k at fp8. Halves QK PE time vs bf16.
  - exp on ACT per [128,1024] psum tile (two heads), mask-mul on DVE with a
    stride-0 broadcast of the keepT slice, PV in bf16 with the
    ones-augmented V (row 64 = softmax denominator).
  - keepT streams per j-column-block (4 x 2MB DMAs) so the first block's
    masks arrive JIT; xT streams per 512-column window; projections beyond
    the first k/q window ride in PE slack during attention via a
    deadline-driven filler queue.
"""

import numpy as np

B, S, D, H = 2, 2048, 1024, 16
HD = D // H  # 64
HEADS_PER_CORE = 4
COLS = HEADS_PER_CORE * HD  # 256
N_CORES = 8
KT = D // 128  # 8 contraction tiles for projections
ST = S // 128  # 16 s tiles
NW = 4  # 512-wide windows
SCALE = 1.0 / np.sqrt(np.float32(D))

_cache = {}


def _build_nc():
    import concourse.bass as bass
    import concourse.mybir as mybir
    import concourse.tile as tile
    from concourse.masks import make_identity

    f32 = mybir.dt.float32
    bf16 = mybir.dt.bfloat16
    f8 = mybir.dt.float8e4
    DR = mybir.MatmulPerfMode.DoubleRow
    ADD = mybir.AluOpType.add
    SUB = mybir.AluOpType.subtract

    nc = bass.Bass(trn_type="TRN2")

    xT = nc.dram_tensor("xT", [D, S], bf16, kind="ExternalInput")
    wq = nc.dram_tensor("wq", [D, COLS], bf16, kind="ExternalInput")
    wk = nc.dram_tensor("wk", [D, COLS], bf16, kind="ExternalInput")
    wv = nc.dram_tensor("wv", [D, COLS], bf16, kind="ExternalInput")
    bq = nc.dram_tensor("bq", [128, 2], f32, kind="ExternalInput")
    bk = nc.dram_tensor("bk", [128, 2], f32, kind="ExternalInput")
    bv = nc.dram_tensor("bv", [1, COLS], bf16, kind="ExternalInput")
    keepT = nc.dram_tensor("keepT", [S, S], bf16, kind="ExternalInput")
    o = nc.dram_tensor("o", [S, COLS], f32, kind="ExternalOutput")

    with tile.TileContext(nc) as tc:
        with (
            tc.tile_pool(name="singles", bufs=1) as singles,
            tc.tile_pool(name="persist", bufs=1) as persist,
            tc.tile_pool(name="big_ps", bufs=2, space="PSUM") as big_ps,
            tc.tile_pool(name="pv_ps", bufs=2, space="PSUM") as pv_ps,
            tc.tile_pool(name="proj_ps", bufs=1, space="PSUM") as proj_ps,
            tc.tile_pool(name="tr_ps", bufs=1, space="PSUM") as tr_ps,
            tc.tile_pool(name="expw", bufs=4) as expw_pool,
            tc.tile_pool(name="expw2", bufs=4) as expw2_pool,
            tc.tile_pool(name="tails", bufs=4) as tails,
        ):
            # ---- constants ----
            ones_col = singles.tile([1, 128], bf16)
            nc.vector.memset(ones_col, 1.0)
            identity = singles.tile([128, 128], f32)
            make_identity(nc, identity)
            bq_sb = singles.tile([128, 2], f32)
            bk_sb = singles.tile([128, 2], f32)
            bv_sb = singles.tile([1, COLS], bf16)

            # ---- persistent buffers ----
            wq_sb = persist.tile([128, KT, COLS], bf16)
            wk_sb = persist.tile([128, KT, COLS], bf16)
            wv_sb = persist.tile([128, KT, COLS], bf16)
            xT_sb = persist.tile([128, KT, S], bf16)
            keepT_sb = persist.tile([128, ST, S], bf16)
            # q/k in fp8 DoubleRow layout: [128p (2 heads x 64 hd), blk, slab, S]
            # blk = head pair (hp); slab 0 = base fp8, slab 1 = residual (q)
            # or duplicate (k).
            qT8 = persist.tile([128, 2, 2, S], f8)
            kT8 = persist.tile([128, 2, 2, S], f8)
            v_aug = persist.tile([128, ST, HEADS_PER_CORE, HD + 1], bf16)
            nc.vector.memset(v_aug[:, :, :, HD : HD + 1], 1.0)

            # ---- DMA issue (order = priority) ----
            xT_r = xT[:, :].rearrange("(kt p) s -> p kt s", p=128)
            keepT_r = keepT[:, :].rearrange("(i p) s -> p i s", p=128)

            def dma_xT_w(w):
                for half in range(2):
                    nc.sync.dma_start(
                        out=xT_sb[:, 4 * half : 4 * half + 4, w * 512 : (w + 1) * 512],
                        in_=xT_r[:, 4 * half : 4 * half + 4, w * 512 : (w + 1) * 512],
                    )

            def dma_keep_j(j, i0, i1, eng):
                eng.dma_start(
                    out=keepT_sb[:, i0:i1, j * 512 : (j + 1) * 512],
                    in_=keepT_r[:, i0:i1, j * 512 : (j + 1) * 512],
                )

            def dma_w(w_sb, w_dram):
                nc.sync.dma_start(
                    out=w_sb,
                    in_=w_dram[:, :].rearrange("(kt p) c -> p kt c", p=128),
                )

            # keepT j0 on the ACT hwdge queue (idle early), everything else
            # on the SP queue; biases after the first xT window (needed only
            # at the first eviction).
            dma_keep_j(0, 0, 8, nc.scalar)
            dma_keep_j(0, 8, 16, nc.scalar)
            dma_w(wk_sb, wk)
            dma_xT_w(0)
            dma_w(wq_sb, wq)
            nc.sync.dma_start(out=bk_sb, in_=bk[:, :])
            nc.sync.dma_start(out=bq_sb, in_=bq[:, :])
            dma_w(wv_sb, wv)
            nc.sync.dma_start(out=bv_sb, in_=bv[:, :])
            dma_xT_w(1)
            dma_xT_w(2)
            dma_xT_w(3)
            dma_keep_j(1, 0, 16, nc.sync)
            dma_keep_j(2, 0, 16, nc.sync)
            dma_keep_j(3, 0, 16, nc.sync)

            # ---- projection groups ----
            def proj_qk(which, blk, w):
                """Project q or k for head-pair blk, sq window w; evict to fp8."""
                w_sb, b_sb, dst = (
                    (wq_sb, bq_sb, qT8),
                    (wk_sb, bk_sb, kT8),
                )[which]
                ps = proj_ps.tile([128, 512], f32, tag="proj")
                for kt in range(KT):
                    nc.tensor.matmul(
                        ps,
                        lhsT=w_sb[:, kt, blk * 128 : (blk + 1) * 128],
                        rhs=xT_sb[:, kt, w * 512 : (w + 1) * 512],
                        start=(kt == 0),
                        stop=(kt == KT - 1),
                        skip_group_check=True,
                    )
                sl = slice(w * 512, (w + 1) * 512)
                if which == 0:  # q: base + residual slabs
                    nc.vector.tensor_scalar_add(
                        out=dst[:, blk, 0, sl], in0=ps, scalar1=b_sb[:, blk : blk + 1]
                    )
                    nc.vector.scalar_tensor_tensor(
                        out=dst[:, blk, 1, sl],
                        in0=ps,
                        scalar=b_sb[:, blk : blk + 1],
                        in1=dst[:, blk, 0, sl],
                        op0=ADD,
                        op1=SUB,
                    )
                else:  # k: duplicate slabs
                    for slab in range(2):
                        nc.vector.tensor_scalar_add(
                            out=dst[:, blk, slab, sl],
                            in0=ps,
                            scalar1=b_sb[:, blk : blk + 1],
                        )

            def proj_v(st):
                psv = proj_ps.tile([128, COLS], f32, tag="proj")
                nc.tensor.matmul(
                    psv,
                    lhsT=ones_col[:, :],
                    rhs=bv_sb[:, :],
                    start=True,
                    stop=False,
                    skip_group_check=True,
                )
                for kt in range(KT):
                    nc.tensor.matmul(
                        psv,
                        lhsT=xT_sb[:, kt, st * 128 : (st + 1) * 128],
                        rhs=wv_sb[:, kt, :],
                        start=False,
                        stop=(kt == KT - 1),
                        skip_group_check=True,
                    )
                nc.vector.tensor_copy(
                    out=v_aug[:, st, :, 0:HD],
                    in_=psv.rearrange("p (h d) -> p h d", h=HEADS_PER_CORE),
                )

            # Filler queue: (deadline_slot, thunk). Slot = (hp*4 + j)*16 + i.
            fillers = []
            for w in range(1, NW):
                fillers.append((4 * w, lambda w=w: proj_qk(1, 0, w)))  # k blk0
            for st in range(ST):
                fillers.append((st, lambda st=st: proj_v(st)))
            for w in range(1, NW):
                fillers.append((16 * w, lambda w=w: proj_qk(0, 0, w)))  # q blk0
            for w in range(NW):
                fillers.append((40 + 4 * w, lambda w=w: proj_qk(1, 1, w)))  # k blk1
            for w in range(NW):
                fillers.append((48 + 4 * w, lambda w=w: proj_qk(0, 1, w)))  # q blk1
            fillers.sort(key=lambda t: t[0])

            # ---- attention ----
            def attention_block(hp, j):
                pvs = [
                    pv_ps.tile([HD + 1, 512], f32, tag="pv", name=f"pv{e}")
                    for e in range(2)
                ]
                for i in range(ST):
                    slot = (hp * 4 + j) * 16 + i
                    while fillers and fillers[0][0] <= slot:
                        fillers.pop(0)[1]()
                    lgp = big_ps.tile([128, 1024], f32, tag="big")
                    for e in range(2):
                        po = e * 64
                        nc.tensor.matmul(
                            lgp[:, e * 512 : (e + 1) * 512],
                            lhsT=kT8[po : po + 64, hp, :, i * 128 : (i + 1) * 128],
                            rhs=qT8[po : po + 64, hp, :, j * 512 : (j + 1) * 512],
                            start=True,
                            stop=True,
                            perf_mode=DR,
                            skip_group_check=True,
                        )
                    ex = expw_pool.tile([128, 1024], bf16)
                    nc.scalar.activation(
                        out=ex,
                        in_=lgp,
                        func=mybir.ActivationFunctionType.Exp,
                        scale=float(SCALE),
                    )
                    ex2 = expw2_pool.tile([128, 1024], bf16)
                    k_ap = keepT_sb[:, i, j * 512 : (j + 1) * 512]
                    k_bcast = bass.AP(
                        tensor=k_ap.tensor,
                        offset=k_ap.offset,
                        ap=[k_ap.ap[0], [0, 2], *k_ap.ap[1:]],
                    )
                    nc.vector.tensor_mul(
                        out=ex2.rearrange("p (e n) -> p e n", e=2),
                        in0=ex.rearrange("p (e n) -> p e n", e=2),
                        in1=k_bcast,
                    )
                    for e in range(2):
                        nc.tensor.matmul(
                            pvs[e],
                            lhsT=v_aug[:, i, 2 * hp + e, :],
                            rhs=ex2[:, e * 512 : (e + 1) * 512],
                            start=(i == 0),
                            stop=(i == ST - 1),
                            skip_group_check=True,
                        )
                # tail: evict both heads, then transpose/normalize/store
                pv_sbs = []
                for e in range(2):
                    pv_sb = tails.tile(
                        [HD + 1, 512], f32, tag="pvsb", name=f"pv_sb{e}"
                    )
                    nc.vector.tensor_copy(out=pv_sb, in_=pvs[e])
                    pv_sbs.append(pv_sb)
                for e in range(2):
                    h = 2 * hp + e
                    pv_sb = pv_sbs[e]
                    ob = tails.tile([128, 4, HD], f32, tag="ob")
                    tr = tr_ps.tile([128, 4, HD + 1], f32, tag="tr")
                    for c in range(4):
                        nc.tensor.transpose(
                            out=tr[:, c, :],
                            in_=pv_sb[:, c * 128 : (c + 1) * 128],
                            identity=identity[0 : HD + 1, 0 : HD + 1],
                        )
                    rc = tails.tile([128, 4], f32, tag="rc")
                    nc.vector.reciprocal(out=rc, in_=tr[:, :, HD : HD + 1])
                    rc_ap = rc[:, :]
                    rc_bcast = bass.AP(
                        tensor=rc_ap.tensor,
                        offset=rc_ap.offset,
                        ap=[*rc_ap.ap, [0, HD]],
                    )
                    nc.vector.tensor_mul(
                        out=ob, in0=tr[:, :, 0:HD], in1=rc_bcast
                    )
                    nc.sync.dma_start(
                        out=o[
                            j * 512 : (j + 1) * 512, h * HD : (h + 1) * HD
                        ].rearrange("(c p) d -> p c d", p=128),
                        in_=ob,
                    )

            # Preamble: first k/q windows so attention starts immediately.
            proj_qk(1, 0, 0)  # k blk0 w0
            proj_qk(0, 0, 0)  # q blk0 w0
            for hp in range(2):
                for j in range(NW):
                    attention_block(hp, j)
            # safety: drain any remaining fillers (shouldn't happen)
            while fillers:
                fillers.pop(0)[1]()

    _split_multiwait(nc)
    return nc


def _split_multiwait(nc, max_waits: int = 1):
    import concourse.mybir as mybir

    for f in nc.m.functions:
        for blk in f.blocks:
            out = []
            changed = False
            for inst in blk.instructions:
                si = inst.sync_info
                if si is not None and len(si.on_wait) > max_waits:
                    waits = list(si.on_wait)
                    extra = waits[: len(waits) - max_waits]
                    keep = waits[len(waits) - max_waits :]
                    for k, w in enumerate(extra):
                        out.append(
                            mybir.InstNoOp(
                                name=f"{inst.name}-wfx{k}",
                                engine=inst.engine,
                                sync_info=mybir.SyncInfo(on_wait=[w], on_update=[]),
                                bass_nofuse=True,
                            )
                        )
                    inst.sync_info = mybir.SyncInfo(
                        on_wait=keep, on_update=list(si.on_update)
                    )
                    changed = True
                out.append(inst)
            if changed:
                blk.instructions = out
    return nc


def _prep_in_maps(x, mask, Wq, bq, Wk, bk, Wv, bv):
    import ml_dtypes

    bf16 = ml_dtypes.bfloat16
    x = np.asarray(x, np.float32)
    mask = np.asarray(mask, bool)

    xT_b = [np.ascontiguousarray(x[b].T).astype(bf16) for b in range(B)]
    keepT_b = [
        np.ascontiguousarray((~mask[b, 0]).T).astype(bf16) for b in range(B)
    ]
    WqT = np.asarray(Wq, np.float32).T.astype(bf16)
    WkT = np.asarray(Wk, np.float32).T.astype(bf16)
    WvT = np.asarray(Wv, np.float32).T.astype(bf16)
    bq32 = np.asarray(bq, np.float32)
    bk32 = np.asarray(bk, np.float32)
    bv = np.asarray(bv, np.float32).astype(bf16)

    in_maps = []
    for c in range(N_CORES):
        b, g = divmod(c, 4)
        cols = slice(g * COLS, (g + 1) * COLS)
        in_maps.append(
            {
                "xT": xT_b[b],
                "wq": np.ascontiguousarray(WqT[:, cols]),
                "wk": np.ascontiguousarray(WkT[:, cols]),
                "wv": np.ascontiguousarray(WvT[:, cols]),
                "bq": np.ascontiguousarray(bq32[cols].reshape(2, 128).T),
                "bk": np.ascontiguousarray(bk32[cols].reshape(2, 128).T),
                "bv": np.ascontiguousarray(bv[cols].reshape(1, COLS)),
                "keepT": keepT_b[b],
            }
        )
    return in_maps


def kernel(x, mask, Wq, bq, Wk, bk, Wv, bv, _trace=False):
    from concourse.bass_utils import run_bass_kernel_spmd

    if "nc" not in _cache:
        _cache["nc"] = _build_nc()
    nc = _cache["nc"]

    in_maps = _prep_in_maps(x, mask, Wq, bq, Wk, bk, Wv, bv)
    res = run_bass_kernel_spmd(
        nc, in_maps, core_ids=list(range(N_CORES)), trace=_trace
    )
    _cache["last_result"] = res

    out = np.empty((B, S, D), np.float32)
    for c in range(N_CORES):
        b, g = divmod(c, 4)
        out[b, :, g * COLS : (g + 1) * COLS] = res.results[c]["o"]
    return out


# revision 10
# speedup vs baseline: 1.0240x; 1.0240x over previous
"""Multi-head attention (B=2, S=2048, D=1024, H=16) on 8 Trainium2 cores.

Sharding: core c handles batch b = c//4 and head group g = c%4 (4 heads,
256 of the 1024 QKV output columns).

v2 design (ACT-bound target):
  - QKV projections in bf16 (lhsT = W.T column slice, rhs = x.T windows of
    512). q/k psums are evicted on DVE as fp8(e4m3) with the bias folded in
    (tensor_scalar_add), PLUS an fp8 residual slab for q computed in one
    scalar_tensor_tensor: r8 = (psum + bias) - q8. k8 is written twice
    (two identical slabs).
  - QK logits use the fp8 DoubleRow perf mode: one matmul per head per
    (i, j) tile contracts both slabs at 0.5 cycles/row:
    logitsT = k8.T @ q8 + k8.T @ r8 = k8.T @ (q8 + r8) — q at ~fp16
    precision, k at fp8. Halves QK PE time vs bf16.
  - exp on ACT per [128,1024] psum tile (two heads), mask-mul on DVE with a
    stride-0 broadcast of the keepT slice, PV in bf16 with the
    ones-augmented V (row 64 = softmax denominator).
  - keepT streams per j-column-block (4 x 2MB DMAs) so the first block's
    masks arrive JIT; xT streams per 512-column window; projections beyond
    the first k/q window ride in PE slack during attention via a
    deadline-driven filler queue.
"""

import numpy as np

B, S, D, H = 2, 2048, 1024, 16
HD = D // H  # 64
HEADS_PER_CORE = 4
COLS = HEADS_PER_CORE * HD  # 256
N_CORES = 8
KT = D // 128  # 8 contraction tiles for projections
ST = S // 128  # 16 s tiles
NW = 4  # 512-wide windows
SCALE = 1.0 / np.sqrt(np.float32(D))

_cache = {}


def _build_nc():
    import concourse.bass as bass
    import concourse.mybir as mybir
    import concourse.tile as tile
    from concourse.masks import make_identity

    f32 = mybir.dt.float32
    bf16 = mybir.dt.bfloat16
    f8 = mybir.dt.float8e4
    DR = mybir.MatmulPerfMode.DoubleRow
    ADD = mybir.AluOpType.add
    SUB = mybir.AluOpType.subtract

    nc = bass.Bass(trn_type="TRN2")

    xT = nc.dram_tensor("xT", [D, S], bf16, kind="ExternalInput")
    wq = nc.dram_tensor("wq", [D, COLS], bf16, kind="ExternalInput")
    wk = nc.dram_tensor("wk", [D, COLS], bf16, kind="ExternalInput")
    wv = nc.dram_tensor("wv", [D, COLS], bf16, kind="ExternalInput")
    bq = nc.dram_tensor("bq", [128, 2], f32, kind="ExternalInput")
    bk = nc.dram_tensor("bk", [128, 2], f32, kind="ExternalInput")
    bv = nc.dram_tensor("bv", [1, COLS], bf16, kind="ExternalInput")
    keepT = nc.dram_tensor("keepT", [S, S], bf16, kind="ExternalInput")
    o = nc.dram_tensor("o", [S, COLS], f32, kind="ExternalOutput")

    with tile.TileContext(nc) as tc:
        with (
            tc.tile_pool(name="singles", bufs=1) as singles,
            tc.tile_pool(name="persist", bufs=1) as persist,
            tc.tile_pool(name="big_ps", bufs=2, space="PSUM") as big_ps,
            tc.tile_pool(name="pv_ps", bufs=2, space="PSUM") as pv_ps,
            tc.tile_pool(name="proj_ps", bufs=1, space="PSUM") as proj_ps,
            tc.tile_pool(name="tr_ps", bufs=1, space="PSUM") as tr_ps,
            tc.tile_pool(name="expw", bufs=4) as expw_pool,
            tc.tile_pool(name="expw2", bufs=4) as expw2_pool,
            tc.tile_pool(name="tails", bufs=4) as tails,
        ):
            # ---- constants ----
            ones_col = singles.tile([1, 128], bf16)
            nc.vector.memset(ones_col, 1.0)
            identity = singles.tile([128, 128], f32)
            make_identity(nc, identity)
            bq_sb = singles.tile([128, 2], f32)
            bk_sb = singles.tile([128, 2], f32)
            bv_sb = singles.tile([1, COLS], bf16)

            # ---- persistent buffers ----
            wq_sb = persist.tile([128, KT, COLS], bf16)
            wk_sb = persist.tile([128, KT, COLS], bf16)
            wv_sb = persist.tile([128, KT, COLS], bf16)
            xT_sb = persist.tile([128, KT, S], bf16)
            keepT_sb = persist.tile([128, ST, S], bf16)
            # q/k in fp8 DoubleRow layout: [128p (2 heads x 64 hd), blk, slab, S]
            # blk = head pair (hp); slab 0 = base fp8, slab 1 = residual (q)
            # or duplicate (k).
            qT8 = persist.tile([128, 2, 2, S], f8)
            kT8 = persist.tile([128, 2, 2, S], f8)
            v_aug = persist.tile([128, ST, HEADS_PER_CORE, HD + 1], bf16)
            nc.vector.memset(v_aug[:, :, :, HD : HD + 1], 1.0)

            # ---- DMA issue (order = priority) ----
            xT_r = xT[:, :].rearrange("(kt p) s -> p kt s", p=128)
            keepT_r = keepT[:, :].rearrange("(i p) s -> p i s", p=128)

            def dma_xT_w(w):
                nc.sync.dma_start(
                    out=xT_sb[:, :, w * 512 : (w + 1) * 512],
                    in_=xT_r[:, :, w * 512 : (w + 1) * 512],
                )

            def dma_keep_j(j, i0, i1, eng):
                eng.dma_start(
                    out=keepT_sb[:, i0:i1, j * 512 : (j + 1) * 512],
                    in_=keepT_r[:, i0:i1, j * 512 : (j + 1) * 512],
                )

            def dma_w(w_sb, w_dram):
                nc.sync.dma_start(
                    out=w_sb,
                    in_=w_dram[:, :].rearrange("(kt p) c -> p kt c", p=128),
                )

            # keepT j0 on the ACT hwdge queue (idle early), everything else
            # on the SP queue; biases after the first xT window (needed only
            # at the first eviction).
            dma_keep_j(0, 0, 8, nc.scalar)
            dma_keep_j(0, 8, 16, nc.scalar)
            dma_w(wk_sb, wk)
            dma_xT_w(0)
            dma_w(wq_sb, wq)
            nc.sync.dma_start(out=bk_sb, in_=bk[:, :])
            nc.sync.dma_start(out=bq_sb, in_=bq[:, :])
            dma_xT_w(1)
            dma_w(wv_sb, wv)
            nc.sync.dma_start(out=bv_sb, in_=bv[:, :])
            dma_xT_w(2)
            dma_xT_w(3)
            dma_keep_j(1, 0, 16, nc.sync)
            dma_keep_j(2, 0, 16, nc.sync)
            dma_keep_j(3, 0, 16, nc.sync)

            # ---- projection groups ----
            def proj_qk(which, blk, w):
                """Project q or k for head-pair blk, sq window w; evict to fp8."""
                w_sb, b_sb, dst = (
                    (wq_sb, bq_sb, qT8),
                    (wk_sb, bk_sb, kT8),
                )[which]
                ps = proj_ps.tile([128, 512], f32, tag="proj")
                for kt in range(KT):
                    nc.tensor.matmul(
                        ps,
                        lhsT=w_sb[:, kt, blk * 128 : (blk + 1) * 128],
                        rhs=xT_sb[:, kt, w * 512 : (w + 1) * 512],
                        start=(kt == 0),
                        stop=(kt == KT - 1),
                        skip_group_check=True,
                    )
                sl = slice(w * 512, (w + 1) * 512)
                if which == 0:  # q: base + residual slabs
                    nc.vector.tensor_scalar_add(
                        out=dst[:, blk, 0, sl], in0=ps, scalar1=b_sb[:, blk : blk + 1]
                    )
                    nc.vector.scalar_tensor_tensor(
                        out=dst[:, blk, 1, sl],
                        in0=ps,
                        scalar=b_sb[:, blk : blk + 1],
                        in1=dst[:, blk, 0, sl],
                        op0=ADD,
                        op1=SUB,
                    )
                else:  # k: duplicate slabs
                    for slab in range(2):
                        nc.vector.tensor_scalar_add(
                            out=dst[:, blk, slab, sl],
                            in0=ps,
                            scalar1=b_sb[:, blk : blk + 1],
                        )

            def proj_v(st):
                psv = proj_ps.tile([128, COLS], f32, tag="proj")
                nc.tensor.matmul(
                    psv,
                    lhsT=ones_col[:, :],
                    rhs=bv_sb[:, :],
                    start=True,
                    stop=False,
                    skip_group_check=True,
                )
                for kt in range(KT):
                    nc.tensor.matmul(
                        psv,
                        lhsT=xT_sb[:, kt, st * 128 : (st + 1) * 128],
                        rhs=wv_sb[:, kt, :],
                        start=False,
                        stop=(kt == KT - 1),
                        skip_group_check=True,
                    )
                nc.vector.tensor_copy(
                    out=v_aug[:, st, :, 0:HD],
                    in_=psv.rearrange("p (h d) -> p h d", h=HEADS_PER_CORE),
                )

            # Filler queue: (deadline_slot, thunk). Slot = (hp*4 + j)*16 + i.
            # Fillers run AFTER the slot's QK (and before its PV), so a
            # k-window feeding QK(i=4w) must land by slot 4w-1, while v(st)
            # feeding PV(st) may land at slot st.
            fillers = []
            for w in range(1, NW):
                fillers.append((4 * w - 1, lambda w=w: proj_qk(1, 0, w)))
            for st in range(ST):
                fillers.append((st, lambda st=st: proj_v(st)))
            for w in range(1, NW):
                fillers.append((16 * w - 1, lambda w=w: proj_qk(0, 0, w)))
            for w in range(NW):
                fillers.append((40 + 4 * w, lambda w=w: proj_qk(1, 1, w)))
            for w in range(NW):
                fillers.append((48 + 4 * w, lambda w=w: proj_qk(0, 1, w)))
            fillers.sort(key=lambda t: t[0])

            def make_drain(hp, j, pvs):
                def drain():
                    pv_sbs = []
                    for e in range(2):
                        pv_sb = tails.tile(
                            [HD + 1, 512], f32, tag="pvsb", name=f"pv_sb{e}"
                        )
                        nc.vector.tensor_copy(out=pv_sb, in_=pvs[e])
                        pv_sbs.append(pv_sb)
                    for e in range(2):
                        h = 2 * hp + e
                        pv_sb = pv_sbs[e]
                        ob = tails.tile([128, 4, HD], f32, tag="ob")
                        tr = tr_ps.tile([128, 4, HD + 1], f32, tag="tr")
                        for c in range(4):
                            nc.tensor.transpose(
                                out=tr[:, c, :],
                                in_=pv_sb[:, c * 128 : (c + 1) * 128],
                                identity=identity[0 : HD + 1, 0 : HD + 1],
                            )
                        rc = tails.tile([128, 4], f32, tag="rc")
                        nc.vector.reciprocal(out=rc, in_=tr[:, :, HD : HD + 1])
                        rc_ap = rc[:, :]
                        rc_bcast = bass.AP(
                            tensor=rc_ap.tensor,
                            offset=rc_ap.offset,
                            ap=[*rc_ap.ap, [0, HD]],
                        )
                        nc.vector.tensor_mul(
                            out=ob, in0=tr[:, :, 0:HD], in1=rc_bcast
                        )
                        nc.sync.dma_start(
                            out=o[
                                j * 512 : (j + 1) * 512, h * HD : (h + 1) * HD
                            ].rearrange("(c p) d -> p c d", p=128),
                            in_=ob,
                        )

                return drain

            # Preamble: first k/q windows so attention starts immediately.
            proj_qk(1, 0, 0)  # k blk0 w0
            proj_qk(0, 0, 0)  # q blk0 w0

            pending_drain = None
            pvs = None
            for slot in range(2 * NW * ST):
                hp, rem = divmod(slot, NW * ST)
                j, i = divmod(rem, ST)
                # QK + exp + mask first: keeps ACT fed across block seams.
                lgp = big_ps.tile([128, 1024], f32, tag="big")
                for e in range(2):
                    po = e * 64
                    nc.tensor.matmul(
                        lgp[:, e * 512 : (e + 1) * 512],
                        lhsT=kT8[po : po + 64, hp, :, i * 128 : (i + 1) * 128],
                        rhs=qT8[po : po + 64, hp, :, j * 512 : (j + 1) * 512],
                        start=True,
                        stop=True,
                        perf_mode=DR,
                        skip_group_check=True,
                    )
                ex = expw_pool.tile([128, 1024], bf16)
                nc.scalar.activation(
                    out=ex,
                    in_=lgp,
                    func=mybir.ActivationFunctionType.Exp,
                    scale=float(SCALE),
                )
                ex2 = expw2_pool.tile([128, 1024], bf16)
                k_ap = keepT_sb[:, i, j * 512 : (j + 1) * 512]
                k_bcast = bass.AP(
                    tensor=k_ap.tensor,
                    offset=k_ap.offset,
                    ap=[k_ap.ap[0], [0, 2], *k_ap.ap[1:]],
                )
                nc.vector.tensor_mul(
                    out=ex2.rearrange("p (e n) -> p e n", e=2),
                    in0=ex.rearrange("p (e n) -> p e n", e=2),
                    in1=k_bcast,
                )
                if i == 0:
                    if pending_drain is not None:
                        pending_drain()
                        pending_drain = None
                while fillers and fillers[0][0] <= slot:
                    fillers.pop(0)[1]()
                if i == 0:
                    pvs = [
                        pv_ps.tile([HD + 1, 512], f32, tag="pv", name=f"pv{e}")
                        for e in range(2)
                    ]
                for e in range(2):
                    nc.tensor.matmul(
                        pvs[e],
                        lhsT=v_aug[:, i, 2 * hp + e, :],
                        rhs=ex2[:, e * 512 : (e + 1) * 512],
                        start=(i == 0),
                        stop=(i == ST - 1),
                        skip_group_check=True,
                    )
                if i == ST - 1:
                    pending_drain = make_drain(hp, j, pvs)
            pending_drain()

    _split_multiwait(nc)
    return nc


def _split_multiwait(nc, max_waits: int = 1):
    import concourse.mybir as mybir

    for f in nc.m.functions:
        for blk in f.blocks:
            out = []
            changed = False
            for inst in blk.instructions:
                si = inst.sync_info
                if si is not None and len(si.on_wait) > max_waits:
                    waits = list(si.on_wait)
                    extra = waits[: len(waits) - max_waits]
                    keep = waits[len(waits) - max_waits :]
                    for k, w in enumerate(extra):
                        out.append(
                            mybir.InstNoOp(
                                name=f"{inst.name}-wfx{k}",
                                engine=inst.engine,
                                sync_info=mybir.SyncInfo(on_wait=[w], on_update=[]),
                                bass_nofuse=True,
                            )
                        )
                    inst.sync_info = mybir.SyncInfo(
                        on_wait=keep, on_update=list(si.on_update)
                    )
                    changed = True
                out.append(inst)
            if changed:
                blk.instructions = out
    return nc


def _prep_in_maps(x, mask, Wq, bq, Wk, bk, Wv, bv):
    import ml_dtypes

    bf16 = ml_dtypes.bfloat16
    x = np.asarray(x, np.float32)
    mask = np.asarray(mask, bool)

    xT_b = [np.ascontiguousarray(x[b].T).astype(bf16) for b in range(B)]
    keepT_b = [
        np.ascontiguousarray((~mask[b, 0]).T).astype(bf16) for b in range(B)
    ]
    WqT = np.asarray(Wq, np.float32).T.astype(bf16)
    WkT = np.asarray(Wk, np.float32).T.astype(bf16)
    WvT = np.asarray(Wv, np.float32).T.astype(bf16)
    bq32 = np.asarray(bq, np.float32)
    bk32 = np.asarray(bk, np.float32)
    bv = np.asarray(bv, np.float32).astype(bf16)

    in_maps = []
    for c in range(N_CORES):
        b, g = divmod(c, 4)
        cols = slice(g * COLS, (g + 1) * COLS)
        in_maps.append(
            {
                "xT": xT_b[b],
                "wq": np.ascontiguousarray(WqT[:, cols]),
                "wk": np.ascontiguousarray(WkT[:, cols]),
                "wv": np.ascontiguousarray(WvT[:, cols]),
                "bq": np.ascontiguousarray(bq32[cols].reshape(2, 128).T),
                "bk": np.ascontiguousarray(bk32[cols].reshape(2, 128).T),
                "bv": np.ascontiguousarray(bv[cols].reshape(1, COLS)),
                "keepT": keepT_b[b],
            }
        )
    return in_maps


def kernel(x, mask, Wq, bq, Wk, bk, Wv, bv, _trace=False):
    from concourse.bass_utils import run_bass_kernel_spmd

    if "nc" not in _cache:
        _cache["nc"] = _build_nc()
    nc = _cache["nc"]

    in_maps = _prep_in_maps(x, mask, Wq, bq, Wk, bk, Wv, bv)
    res = run_bass_kernel_spmd(
        nc, in_maps, core_ids=list(range(N_CORES)), trace=_trace
    )
    _cache["last_result"] = res

    out = np.empty((B, S, D), np.float32)
    for c in range(N_CORES):
        b, g = divmod(c, 4)
        out[b, :, g * COLS : (g + 1) * COLS] = res.results[c]["o"]
    return out
